# revision 16
# baseline (speedup 1.0000x reference)
"""Trainium2 Bass kernel for the Mahalanobis loss:

    out = mean_b( sqrt( delta[b] @ S_inv @ delta[b] ) ),  delta = original - reconstruction

Full shapes: original/reconstruction [8192, 2048] f32, S_inv [2048, 2048] f32.
Data-parallel over batch on 8 NeuronCores: core i handles rows [i*1024,(i+1)*1024).

v15 (v12 88us -> v13 64.6 -> v14 66.7 -> here).  Key discoveries driving
this design (hw-measured):
  - tc.For_i places an ALL-ENGINE BARRIER per iteration, so per-iteration
    time is the full serial critical path.  v15 unrolls U=16 logical bodies
    per For_i iteration; bodies pipeline against each other through the
    tile scheduler (pool-rotated double buffers), so fill/drain and the
    barrier amortize 16x (measured: U=8 50.8us, U=16 48.9us per body).
  - fp8 elementwise rates: DVE tensor_tensor 1.28us per [128,1024] block,
    Pool 2.03us (0.42 efficiency), no fp8 packing modes.  DVE therefore
    runs ONLY the 16 column products (PSUM f32 x fp8 -> fp8, one fused
    [128,1024] op per close); Pool subtracts 6 delta blocks; the middle 10
    delta blocks are produced by gpsimd software-DGE accum DMAs
    (d8 = orig; d8 += -recon), which run on otherwise-idle DMA engines at
    ~1.3us/block.  Accum chains are limited to 2 KiB/partition (2 blocks):
    larger accum DMAs die with NRT_EXEC_UNIT_UNRECOVERABLE.
  - S is uploaded fp8 pre-masked as M2 (M2 + M2' = 2*S, split diagonal =>
    all-uniform fp8 DoubleRow pairs, 72 cells) and stays RESIDENT in SBUF
    across the loop (loaded once, 18 KiB/partition).
  - q reduce: h0 via paired fp8 DoubleRow ones-matmuls into PSUM rows 0-15
    (DR + col-tiling fails the walrus ISA check, so col-group 0 only), h1
    via normal-mode fp8 ones-matmuls into row 32 of the same bank.
  - PSUM: 2 qps banks (body-alternating) + a 3-deep ring of 2-bank column
    accumulators shared by all columns of all bodies = 8 banks.

Engine budget per body: DVE ~20us, PE ~20.5us, Pool ~13us, accum lane
~13us, DMA ~7us -- steady-state target ~22-24us/iteration.
"""

import numpy as np

P = 128
B_FULL, D = 8192, 2048
N_CORES = 8
B_SH = B_FULL // N_CORES    # 1024
NJ = D // P                 # 16 d/e blocks
U = 16                      # bodies per For_i iteration (barrier amortization)

# delta production lanes
SUB_BLOCKS = [0, 1, 2, 3, 14, 15]      # Pool tensor_tensor subtract
ACC_BLOCKS = list(range(4, 14))        # DMA accum chains (o then -r)
X_ORDER = SUB_BLOCKS + ACC_BLOCKS      # block order inside x_o / x_r
NSUB = len(SUB_BLOCKS)

# column c owns cb(c) = 2*(c//2) + 2 blocks (full DoubleRow pairs only)
CB = [2 * (c // 2) + 2 for c in range(NJ)]
S_BOFF = [0]
for c in range(NJ):
    S_BOFF.append(S_BOFF[-1] + CB[c])
NBLK_TOT = S_BOFF[-1]       # 144
SG = [(4 * g, S_BOFF[4 * g], S_BOFF[4 * g + 4]) for g in range(4)]

_CACHED = {}


def _build(b_sh=B_SH, d=D, loop=1):
    import contextlib

    import concourse.tile as tile
    from concourse import bacc, mybir

    nc = bacc.Bacc("TRN2", target_bir_lowering=False)
    f32 = mybir.dt.float32
    fp8 = mybir.dt.float8e4
    DR = mybir.MatmulPerfMode.DoubleRow

    # [p, block (X_ORDER), half, 512]; x_r's ACC half is pre-negated
    x_o = nc.dram_tensor("x_o", [P, NJ, 2, 512], fp8, kind="ExternalInput")
    x_r = nc.dram_tensor("x_r", [P, NJ, 2, 512], fp8, kind="ExternalInput")
    s_gs = [nc.dram_tensor(f"s_g{gi}", [P, (b1 - b0) * P], fp8,
                           kind="ExternalInput")
            for gi, (_, b0, b1) in enumerate(SG)]
    q_out = nc.dram_tensor("q_out", [1, 2 * U], f32, kind="ExternalOutput")

    with tile.TileContext(nc) as tc:
        with (
            tc.tile_pool(name="sbf", bufs=1) as s_pool,
            tc.tile_pool(name="dd", bufs=2) as d_pool,
            tc.tile_pool(name="pr", bufs=3) as pr_pool,
            tc.tile_pool(name="cst", bufs=1) as cst_pool,
            tc.tile_pool(name="tail", bufs=2) as tail_pool,
            tc.tile_pool(name="psq", bufs=2, space="PSUM") as psq_pool,
            tc.tile_pool(name="pscol", bufs=3, space="PSUM") as ps_pool,
        ):
            # --- loop-invariant: constants + resident S ---
            ones2 = cst_pool.tile([P, 2, 16], fp8, name="ones2", tag="ones2")
            nc.vector.memset(ones2[:], 1.0)
            ones1 = cst_pool.tile([P, 1], fp8, name="ones1", tag="ones1")
            nc.vector.memset(ones1[:], 1.0)
            s8 = s_pool.tile([P, NBLK_TOT, P], fp8, name="s8", tag="s8")
            for gi, (_, b0, b1) in enumerate(SG):
                nc.scalar.dma_start(s8[:, b0:b1, :], s_gs[gi][:])

            def body(bi):
                """One logical iteration; bi indexes the q_out slot."""
                o8 = d_pool.tile([P, NSUB, 2, 512], fp8, name=f"o8_{bi}",
                                 tag="o8")
                r8 = d_pool.tile([P, NSUB, 2, 512], fp8, name=f"r8_{bi}",
                                 tag="r8")
                d8 = d_pool.tile([P, NJ, 2, 512], fp8, name=f"d8_{bi}",
                                 tag="d8")
                qps2 = psq_pool.tile([48, 512], f32, name=f"qps_{bi}",
                                     tag="qps")

                # x loads: accum-lane o's straight into d8 (one DMA), then
                # 2-block accum chains add -recon; sub-lane o/r into o8/r8.
                # per-chain o-loads so accum chain k only waits on its own
                # 2 blocks (one shared 1.25 MiB load made all 5 r-accum
                # chains serialize behind a single completion semaphore)
                for k in range(5):
                    nc.sync.dma_start(
                        d8[:, 4 + 2 * k:6 + 2 * k, :, :],
                        x_o[:, NSUB + 2 * k:NSUB + 2 * k + 2, :, :])
                nc.sync.dma_start(o8[:], x_o[:, 0:NSUB, :, :])
                nc.scalar.dma_start(r8[:], x_r[:, 0:NSUB, :, :])

                def emit_sub(i):
                    nc.gpsimd.tensor_tensor(
                        d8[:, SUB_BLOCKS[i], :, :], o8[:, i, :, :],
                        r8[:, i, :, :], mybir.AluOpType.subtract)

                # Pool queue: early subs (unblock columns 0-3), then the
                # accum-chain descriptor generations (SEQ-side, overlap the
                # sub TTs on the engine), then the late subs (blocks 14,15).
                for i in range(4):
                    emit_sub(i)
                for k in range(5):
                    nc.gpsimd.dma_start(
                        d8[:, 4 + 2 * k:6 + 2 * k, :, :],
                        x_r[:, NSUB + 2 * k:NSUB + 2 * k + 2, :, :],
                        accum_op=mybir.AluOpType.add)
                emit_sub(4)
                emit_sub(5)

                n_closed = 0
                pr = None
                for m in range(NJ // 2):        # dpair index
                    for c in (2 * m, 2 * m + 1):
                        ph = ps_pool.tile([P, 2, 512], f32,
                                          name=f"ps_{bi}_{c}", tag="ps")
                        for mm in range(c // 2 + 1):
                            lhsT = s8[:, S_BOFF[c] + 2 * mm:
                                      S_BOFF[c] + 2 * mm + 2, :]
                            for h in range(2):
                                nc.tensor.matmul(
                                    ph[:, h, :], lhsT,
                                    d8[:, 2 * mm:2 * mm + 2, h, :],
                                    start=(mm == 0), stop=(mm == c // 2),
                                    perf_mode=DR)
                        # close: fused [128,1024] product, then reduce
                        slot = n_closed % 2
                        if slot == 0:
                            pr = pr_pool.tile([P, 2, 2, 512], fp8,
                                              name=f"pr_{bi}_{c}", tag="pr")
                        nc.vector.tensor_tensor(
                            pr[:, slot, :, :], ph[:, :, :], d8[:, c, :, :],
                            mybir.AluOpType.mult)
                        nc.tensor.matmul(
                            qps2[32:33, :], ones1[:], pr[:, slot, 1, :],
                            start=(n_closed == 0), stop=(n_closed == NJ - 1),
                            skip_group_check=True)
                        if slot == 1:
                            pi = n_closed // 2
                            nc.tensor.matmul(
                                qps2[0:16, :], ones2[:], pr[:, :, 0, :],
                                start=(pi == 0), stop=(pi == NJ // 2 - 1),
                                perf_mode=DR, skip_group_check=True)
                        n_closed += 1

                # tail: fused sqrt+sum per half
                red = tail_pool.tile([1, 2], f32, name=f"red_{bi}",
                                     tag="red")
                sq = tail_pool.tile([1, b_sh], f32, name=f"sq_{bi}",
                                    tag="sq")
                for h in range(2):
                    nc.scalar.activation(
                        out=sq[:, h * 512:(h + 1) * 512],
                        in_=qps2[32 * h:32 * h + 1, :],
                        func=mybir.ActivationFunctionType.Sqrt,
                        accum_out=red[:, h:h + 1])
                nc.scalar.dma_start(q_out[:, 2 * bi:2 * bi + 2], red[:])

            n_for, rem = divmod(loop, U)
            if n_for == 1:
                rem, n_for = rem + U, 0       # flat, no loop hardware
            if n_for >= 1:
                with tc.For_i(0, n_for, 1):
                    for bi in range(U):
                        body(bi)
            for bi in range(rem):
                body(bi % U)

    nc.compile()
    return nc


def _get_nc():
    if "nc" not in _CACHED:
        _CACHED["nc"] = _build()
    return _CACHED["nc"]


def _np_fp8():
    import ml_dtypes
    return np.dtype(ml_dtypes.float8_e4m3)


def make_in_maps(original, reconstruction, S_inv):
    """Host-side sharding/packing: slicing, transposes, fp8 quantization."""
    f8 = _np_fp8()
    s = np.asarray(S_inv, dtype=np.float32)

    # mask2 = 2*strict_upper + diag (so M2 + M2.T = 2*S on diag blocks)
    mask2 = (2.0 * np.triu(np.ones((P, P), np.float32), 1)
             + np.eye(P, dtype=np.float32))

    def blk(j, c):
        return s[j * P:(j + 1) * P, c * P:(c + 1) * P]

    cols = []
    for c in range(NJ):
        bs = [2.0 * blk(j, c) for j in range(2 * (c // 2))]
        if c % 2 == 0:
            bs += [mask2 * blk(c, c), blk(c + 1, c)]
        else:
            bs += [blk(c - 1, c), mask2 * blk(c, c)]
        cols.append(np.concatenate(bs, axis=1))
    s_groups = {
        f"s_g{g}": np.ascontiguousarray(
            np.concatenate(cols[4 * g:4 * g + 4], axis=1)).astype(f8)
        for g in range(4)}

    perm = np.asarray(X_ORDER)
    neg = np.ones((1, NJ, 1, 1), np.float32)
    neg[0, NSUB:] = -1.0     # accum-lane recon blocks pre-negated

    in_maps = []
    for i in range(N_CORES):
        sl = slice(i * B_SH, (i + 1) * B_SH)
        o = np.asarray(original[sl], np.float32).T      # [D, 1024]
        r = np.asarray(reconstruction[sl], np.float32).T
        # [p, block, half, 512] with blocks permuted to X_ORDER
        ov = o.reshape(NJ, P, 2, 512).transpose(1, 0, 2, 3)[:, perm]
        rv = r.reshape(NJ, P, 2, 512).transpose(1, 0, 2, 3)[:, perm] * neg
        in_maps.append({"x_o": np.ascontiguousarray(ov).astype(f8),
                        "x_r": np.ascontiguousarray(rv).astype(f8),
                        **s_groups})
    return in_maps


def kernel(original: np.ndarray, reconstruction: np.ndarray,
           S_inv: np.ndarray) -> np.ndarray:
    from concourse import bass_utils

    nc = _get_nc()
    in_maps = make_in_maps(original, reconstruction, S_inv)
    res = bass_utils.run_bass_kernel_spmd(
        nc, in_maps, core_ids=list(range(N_CORES)),
        trace=_CACHED.get("trace", False),
    )
    _CACHED["last_results"] = res

    total = sum(float(np.asarray(r["q_out"])[:, 0:2].astype(np.float64).sum())
                for r in res.results)
    return np.float32(total / B_FULL)
